# revision 11
# baseline (speedup 1.0000x reference)
"""Trainium2 Bass kernel for MC2RowParallelLinear: Y = X @ W^T + bias.

Full shapes: X [4096, 2, 8192] f32, W [2048, 8192] f32, bias [2048] f32,
Y [4096, 2, 2048] f32.

Strategy (8 NeuronCores): data-parallel over the sequence dim — each core
owns 512 seq rows (1024 flattened [s,b] rows) and computes its Y shard with
the full weight. No collectives needed; the host gathers shards. Inputs are
pre-transposed on the host into k-major layouts so the device does zero
transposes and every DMA is a contiguous ~1 MiB block.

Device kernel (per core): streaming GEMM over 8 K-passes of 8 k-tiles; the
Y accumulator lives in SBUF across passes so X and W are each read from HBM
exactly once (105 MiB/core, well under the PE roofline). Within a pass,
matmuls are issued row-major across 8 concurrently-open PSUM banks (2 sb
tiles x 4 n blocks), so W rows are consumed in arrival order and the next
pass's panel only needs a partial prefetch. Matmuls run in float32r
(full-rate fp32 on the PE, ~1e-4 max rel err at K=8192); accumulation is
exact fp32 in PSUM/SBUF.
"""

import numpy as np

import concourse.bacc as bacc
import concourse.mybir as mybir
import concourse.tile as tile
from concourse.bass_utils import run_bass_kernel_spmd

S, B, K, N = 4096, 2, 8192, 2048
CORES = 8
SB = S * B           # 8192 flattened rows
SBL = SB // CORES    # 1024 rows per core
P = 128
KT = K // P          # 64 k-tiles
KQ = 8               # k passes (Y_acc += per pass)
KTQ = KT // KQ       # 8 k-tiles per pass = one PSUM accumulation group
ST = SBL // P        # 8 sb tiles per core
G = 2                # sb tiles per X block (1 MiB DMA granularity)
STG = ST // G        # 4 X blocks per (core, k-pass)
NBW = 512            # n block width (one PSUM bank, 4-byte moving-op max)
NB = N // NBW        # 4 n blocks

MDT = mybir.dt.float32r
F32 = mybir.dt.float32

_cache = {}


def build(reps=1):
    """reps>1 wraps the GEMM body in a hardware loop — timing-only variant."""
    import contextlib

    nc = bacc.Bacc(None, target_bir_lowering=False)
    xt = nc.dram_tensor("xt", [KQ, STG, P, KTQ, G * P], MDT, kind="ExternalInput")
    wt = nc.dram_tensor("wt", [KT, P, N], MDT, kind="ExternalInput")
    bias = nc.dram_tensor("bias", [P, N], F32, kind="ExternalInput")
    y = nc.dram_tensor("y", [ST, P, N], F32, kind="ExternalOutput")
    with tile.TileContext(nc) as tc:
        with tc.tile_pool(name="wp", bufs=KTQ + 4) as wp, \
             tc.tile_pool(name="xp", bufs=3) as xp, \
             tc.tile_pool(name="acc", bufs=1) as accp, \
             tc.tile_pool(name="cst", bufs=1) as cst, \
             tc.tile_pool(name="ps", bufs=8, space="PSUM") as psp:
            bias_sb = cst.tile([P, N], F32, tag="bias")
            nc.sync.dma_start(bias_sb[:], bias[:])
            yaccs = [accp.tile([P, N], F32, tag=f"yacc{st}", name=f"yacc{st}")
                     for st in range(ST)]
            loop = tc.For_i(0, reps, 1) if reps > 1 else contextlib.nullcontext()
            with loop:
                _body(nc, wp, xp, psp, xt, wt, y, bias_sb, yaccs)
    nc.compile()
    return nc


def _body(nc, wp, xp, psp, xt, wt, y, bias_sb, yaccs):
    for kq in range(KQ):
        # W rows for this pass: KTQ 1 MiB loads. Pool slack (bufs=KTQ+4)
        # prefetches the next pass's leading rows; the row-major matmul
        # order below consumes rows as they arrive.
        wrows = []
        for ktq in range(KTQ):
            w = wp.tile([P, N], MDT, tag="w", name=f"w_{kq}_{ktq}")
            nc.sync.dma_start(w[:], wt[kq * KTQ + ktq])
            wrows.append(w)
        for stg in range(STG):
            xblk = xp.tile([P, KTQ, G * P], MDT, tag="x", name=f"x_{kq}_{stg}")
            nc.sync.dma_start(xblk[:], xt[kq, stg])
            # Open all G*NB = 8 PSUM groups for this X block, then stream
            # the 8 W rows through them in row-major order.
            pss = [[psp.tile([P, NBW], F32, tag="ps", name=f"ps_{kq}_{stg}_{g}_{nb}")
                    for nb in range(NB)] for g in range(G)]
            for ktq in range(KTQ):
                for g in range(G):
                    for nb in range(NB):
                        nc.tensor.matmul(
                            pss[g][nb][:],
                            xblk[:, ktq, g * P:(g + 1) * P],
                            wrows[ktq][:, nb * NBW:(nb + 1) * NBW],
                            start=(ktq == 0), stop=(ktq == KTQ - 1))
            for g in range(G):
                st = stg * G + g
                for nb in range(NB):
                    ysl = yaccs[st][:, nb * NBW:(nb + 1) * NBW]
                    if kq == 0:
                        nc.vector.tensor_add(
                            ysl, pss[g][nb][:],
                            bias_sb[:, nb * NBW:(nb + 1) * NBW])
                    else:
                        nc.vector.tensor_add(ysl, ysl, pss[g][nb][:])
                if kq == KQ - 1:
                    nc.sync.dma_start(y[st], yaccs[st][:])


def shard_inputs(input_, weight, bias):
    X = np.ascontiguousarray(np.asarray(input_, np.float32)).reshape(SB, K)
    W = np.ascontiguousarray(np.asarray(weight, np.float32))
    b = np.ascontiguousarray(np.asarray(bias, np.float32))
    WT = np.ascontiguousarray(W.T).reshape(KT, P, N)
    bias_rep = np.ascontiguousarray(np.broadcast_to(b, (P, N)))
    in_maps = []
    for c in range(CORES):
        Xl = X[c * SBL:(c + 1) * SBL]
        # row = (stg*G + g)*P + sb, col = (kq*KTQ + ktq)*P + p
        #   -> [kq, stg, p, ktq, g*P + sb]
        xt = np.ascontiguousarray(
            Xl.reshape(STG, G, P, KQ, KTQ, P)
            .transpose(3, 0, 5, 4, 1, 2)
            .reshape(KQ, STG, P, KTQ, G * P))
        in_maps.append({"xt": xt, "wt": WT, "bias": bias_rep})
    return in_maps


def kernel(input_, weight, bias):
    if "nc" not in _cache:
        _cache["nc"] = build()
    nc = _cache["nc"]
    in_maps = shard_inputs(input_, weight, bias)
    res = run_bass_kernel_spmd(nc, in_maps, core_ids=list(range(CORES)))
    out = np.concatenate([r["y"].reshape(SBL, N) for r in res.results], axis=0)
    return out.reshape(S, B, N)
